# revision 4
# baseline (speedup 1.0000x reference)
"""Multi-head causal attention on 8 Trainium2 NeuronCores.

Sharding: tensor-parallel over heads x data-parallel over batch.
Core c handles batch c//4 and heads [4*(c%4), 4*(c%4)+4). Each core
computes Q/K/V projections for its head slice over the full sequence,
causal flash-style attention (transposed scores, ones-column softmax
denominator), and a partial output projection against its row-slice of
W_o. The 4 partial outputs per batch are summed on the host (the
all-reduce of row-parallel W_o), which also adds b_o.
"""
import sys

sys.path.insert(0, '/opt/trn_rl_repo')

import numpy as np
import ml_dtypes

B, S, D, H, DK = 2, 2048, 1024, 16, 64
NCORES = 8
HL = 4            # heads per core
DL = HL * DK      # head-dim slice per core (256)
NQB = S // 512    # 512-wide query blocks
NKST = S // 128   # 128-wide key tiles

_cache = {}


def _build(repeat=1, dynamic=False):
    import concourse.bacc as bacc
    import concourse.mybir as mybir
    import concourse.tile as tile
    from contextlib import ExitStack, nullcontext

    f32, f32r, bf16 = mybir.dt.float32, mybir.dt.float32r, mybir.dt.bfloat16
    Ident, Exp = mybir.ActivationFunctionType.Identity, mybir.ActivationFunctionType.Exp

    nc = bacc.Bacc("TRN2", target_bir_lowering=False, debug=False, num_devices=NCORES)
    xt_d = nc.dram_tensor("xt", (D, S), f32r, kind="ExternalInput").ap()
    wq_d = nc.dram_tensor("wq", (D, DL), f32r, kind="ExternalInput").ap()
    wk_d = nc.dram_tensor("wk", (D, DL), f32r, kind="ExternalInput").ap()
    wv_d = nc.dram_tensor("wv", (D, DL), f32r, kind="ExternalInput").ap()
    wo_d = nc.dram_tensor("wo", (DL, D), f32r, kind="ExternalInput").ap()
    bq_d = nc.dram_tensor("bq", (DL,), f32, kind="ExternalInput").ap()
    bk_d = nc.dram_tensor("bk", (DL,), f32, kind="ExternalInput").ap()
    bv_d = nc.dram_tensor("bv", (DL,), f32, kind="ExternalInput").ap()
    mask_d = nc.dram_tensor("masks", (4, 128, 512), bf16, kind="ExternalInput").ap()
    po_d = nc.dram_tensor("po", (D, S), f32, kind="ExternalOutput").ap()

    with tile.TileContext(nc) as tc:
        with ExitStack() as ctx:
            sb = ctx.enter_context(tc.tile_pool(name="sb", bufs=1))
            ps = ctx.enter_context(tc.tile_pool(name="ps", bufs=1, space="PSUM"))

            # ---- persistent SBUF tiles ----
            xt = [sb.tile([128, S], f32r, name=f"xt{k}") for k in range(8)]
            wq_s = [sb.tile([128, DL], f32r, name=f"wq{k}") for k in range(8)]
            wk_s = [sb.tile([128, DL], f32r, name=f"wk{k}") for k in range(8)]
            wv_s = [sb.tile([128, DL], f32r, name=f"wv{k}") for k in range(8)]
            wo_s = [sb.tile([128, D], f32r, name=f"wo{k}") for k in range(2)]
            qt = [sb.tile([128, S], f32r, name=f"qt{p}") for p in range(2)]
            kt = [sb.tile([128, S], f32r, name=f"kt{p}") for p in range(2)]
            ctxt = [sb.tile([128, S], f32r, name=f"ctxt{p}") for p in range(2)]
            # v_aug: [128, ks-tile, 2 pairs x (64 even | one | 64 odd | spare)]
            v_aug = sb.tile([128, NKST, 264], bf16, name="v_aug")
            masks = sb.tile([128, 4, 512], bf16, name="masks")
            bq_t = [sb.tile([128, 1], f32, name=f"bq{p}") for p in range(2)]
            bk_t = [sb.tile([128, 1], f32, name=f"bk{p}") for p in range(2)]
            bv_sb = sb.tile([1, DL], f32, name="bv_sb")
            bvB = sb.tile([128, DL], f32, name="bvB")

            rep_ctx = tc.For_i(0, repeat, 1) if dynamic else nullcontext(range(repeat))
            with rep_ctx as _it:
              for _rep in ([0] if dynamic else _it):
                # ---- input DMAs ----
                for k in range(8):
                    nc.sync.dma_start(xt[k][:], xt_d[k * 128:(k + 1) * 128, :])
                    nc.sync.dma_start(wq_s[k][:], wq_d[k * 128:(k + 1) * 128, :])
                    nc.sync.dma_start(wk_s[k][:], wk_d[k * 128:(k + 1) * 128, :])
                    nc.sync.dma_start(wv_s[k][:], wv_d[k * 128:(k + 1) * 128, :])
                for k in range(2):
                    nc.sync.dma_start(wo_s[k][:], wo_d[k * 128:(k + 1) * 128, :])
                nc.sync.dma_start(masks[:], mask_d.rearrange("t p n -> p t n"))
                for p in range(2):
                    nc.sync.dma_start(bq_t[p][:], bq_d[p * 128:(p + 1) * 128].rearrange("(p o) -> p o", o=1))
                    nc.sync.dma_start(bk_t[p][:], bk_d[p * 128:(p + 1) * 128].rearrange("(p o) -> p o", o=1))
                nc.sync.dma_start(bv_sb[:], bv_d.rearrange("(o n) -> o n", o=1))
                nc.gpsimd.partition_broadcast(bvB[:], bv_sb[:])

                # ---- K / Q projections: [dout-pair-tile, seq] ----
                for dst, w_s, b_t in ((kt, wk_s, bk_t), (qt, wq_s, bq_t)):
                    for p in range(2):
                        for qb in range(NQB):
                            pp = ps.tile([128, 512], f32, tag="b512", bufs=4)
                            for k in range(8):
                                nc.tensor.matmul(pp[:], w_s[k][:, p * 128:(p + 1) * 128],
                                                 xt[k][:, qb * 512:(qb + 1) * 512],
                                                 start=(k == 0), stop=(k == 7))
                            nc.scalar.activation(dst[p][:, qb * 512:(qb + 1) * 512], pp[:],
                                                 Ident, bias=b_t[p][:])

                # ---- V projection: seq-major + augmented ones columns ----
                for sp in range(NKST):
                    pv = ps.tile([128, 512], f32, tag="b512", bufs=4)
                    for k in range(8):
                        nc.tensor.matmul(pv[:, 0:DL], xt[k][:, sp * 128:(sp + 1) * 128],
                                         wv_s[k][:], start=(k == 0), stop=(k == 7))
                    dst = v_aug[:, sp, :].rearrange("p (pr e q) -> p pr e q", pr=2, e=2, q=66)
                    nc.vector.tensor_add(dst[:, :, :, 0:64],
                                         pv[:, 0:DL].rearrange("p (pr e q) -> p pr e q", pr=2, e=2, q=64),
                                         bvB[:].rearrange("p (pr e q) -> p pr e q", pr=2, e=2, q=64))
                    nc.gpsimd.memset(dst[:, :, :, 64:65], 1.0)

                # ---- causal attention, transposed scores ----
                for p in range(2):
                    for qb in range(NQB):
                        n_kst = 4 * qb + 4
                        n_grp = n_kst // 2
                        av0 = ps.tile([65, 512], f32, tag="b512", bufs=4)
                        av1 = ps.tile([65, 512], f32, tag="b512", bufs=4)
                        for g in range(n_grp):
                            sc0 = ps.tile([128, 1024], f32, tag="sc", bufs=2)
                            sc1 = ps.tile([128, 1024], f32, tag="sc", bufs=2)
                            for i in range(2):
                                kst = 2 * g + i
                                nc.tensor.matmul(sc0[:, i * 512:(i + 1) * 512],
                                                 kt[p][0:64, kst * 128:(kst + 1) * 128],
                                                 qt[p][0:64, qb * 512:(qb + 1) * 512],
                                                 start=True, stop=True, tile_position=(0, 0))
                                nc.tensor.matmul(sc1[:, i * 512:(i + 1) * 512],
                                                 kt[p][64:128, kst * 128:(kst + 1) * 128],
                                                 qt[p][64:128, qb * 512:(qb + 1) * 512],
                                                 start=True, stop=True, tile_position=(64, 0))
                            e0 = sb.tile([128, 1024], bf16, tag="ex", bufs=4)
                            e1 = sb.tile([128, 1024], bf16, tag="ex", bufs=4)
                            nc.scalar.activation(e0[:], sc0[:], Exp, scale=0.125)
                            nc.scalar.activation(e1[:], sc1[:], Exp, scale=0.125)
                            for i in range(2):
                                kst = 2 * g + i
                                mi = kst - 4 * qb
                                if mi >= 0:
                                    sl = slice(i * 512, (i + 1) * 512)
                                    nc.vector.tensor_mul(e0[:, sl], e0[:, sl], masks[:, mi, :])
                                    nc.vector.tensor_mul(e1[:, sl], e1[:, sl], masks[:, mi, :])
                            for i in range(2):
                                kst = 2 * g + i
                                st, sp_ = (g == 0 and i == 0), (g == n_grp - 1 and i == 1)
                                nc.tensor.matmul(av0[:], v_aug[:, kst, p * 132:p * 132 + 65],
                                                 e0[:, i * 512:(i + 1) * 512], start=st, stop=sp_)
                                nc.tensor.matmul(av1[:], v_aug[:, kst, p * 132 + 66:p * 132 + 131],
                                                 e1[:, i * 512:(i + 1) * 512], start=st, stop=sp_)
                        for e, av in ((0, av0), (1, av1)):
                            rc = sb.tile([1, 512], f32, tag="rc", bufs=2)
                            rb = sb.tile([64, 512], f32, tag="rb", bufs=2)
                            nc.vector.reciprocal(rc[:], av[64:65, :])
                            nc.gpsimd.partition_broadcast(rb[:], rc[:])
                            nc.vector.tensor_mul(ctxt[p][e * 64:(e + 1) * 64, qb * 512:(qb + 1) * 512],
                                                 av[0:64, :], rb[:])

                # ---- partial output projection ----
                for qb in range(NQB):
                    for ot in range(8):
                        po_p = ps.tile([128, 512], f32, tag="b512", bufs=4)
                        for k in range(2):
                            nc.tensor.matmul(po_p[:], wo_s[k][:, ot * 128:(ot + 1) * 128],
                                             ctxt[k][:, qb * 512:(qb + 1) * 512],
                                             start=(k == 0), stop=(k == 1))
                        po_sb = sb.tile([128, 512], f32, tag="po_s", bufs=3)
                        if ot % 2 == 0:
                            nc.scalar.activation(po_sb[:], po_p[:], Ident)
                        else:
                            nc.vector.tensor_copy(po_sb[:], po_p[:])
                        nc.sync.dma_start(po_d[ot * 128:(ot + 1) * 128, qb * 512:(qb + 1) * 512], po_sb[:])

    nc.compile()
    return nc


def _causal_mask_ok(mask):
    m = np.asarray(mask)
    if m.shape != (S, S):
        return False
    return np.array_equal(m.astype(bool), np.triu(np.ones((S, S), bool), k=1))


def _numpy_fallback(x, mask, Wq, bq, Wk, bk, Wv, bv, Wo, bo):
    x = np.asarray(x, np.float64)
    q = (x @ Wq + bq).reshape(B, S, H, DK).transpose(0, 2, 1, 3)
    k = (x @ Wk + bk).reshape(B, S, H, DK).transpose(0, 2, 1, 3)
    v = (x @ Wv + bv).reshape(B, S, H, DK).transpose(0, 2, 1, 3)
    s = np.einsum("bhqd,bhkd->bhqk", q, k) / np.sqrt(DK)
    s = np.where(np.asarray(mask, bool), -np.inf, s)
    s = s - s.max(-1, keepdims=True)
    e = np.exp(s)
    a = e / e.sum(-1, keepdims=True)
    ctx = np.einsum("bhqk,bhkd->bhqd", a, v).transpose(0, 2, 1, 3).reshape(B, S, D)
    return (ctx @ Wo + bo).astype(np.float32)


def _tri_masks():
    m = np.zeros((4, 128, 512), np.float32)
    n = np.arange(512)
    for t in range(4):
        for p_ in range(128):
            m[t, p_, :] = (n >= t * 128 + p_)
    return m.astype(ml_dtypes.bfloat16)


def kernel(x, mask, Wq, bq, Wk, bk, Wv, bv, Wo, bo):
    x = np.ascontiguousarray(np.asarray(x, np.float32))
    if not _causal_mask_ok(mask):
        return _numpy_fallback(x, mask, Wq, bq, Wk, bk, Wv, bv, Wo, bo)

    from concourse import bass_utils

    if "nc" not in _cache:
        _cache["nc"] = _build(repeat=1)
    nc = _cache["nc"]

    Wq, Wk, Wv, Wo = (np.asarray(w, np.float32) for w in (Wq, Wk, Wv, Wo))
    bq, bk, bv, bo = (np.asarray(b_, np.float32) for b_ in (bq, bk, bv, bo))
    masks_np = _tri_masks()
    xts = [np.ascontiguousarray(x[b_].T) for b_ in range(B)]

    in_maps = []
    for c in range(NCORES):
        b_, hs = c // 4, (c % 4) * DL
        in_maps.append({
            "xt": xts[b_],
            "wq": np.ascontiguousarray(Wq[:, hs:hs + DL]),
            "wk": np.ascontiguousarray(Wk[:, hs:hs + DL]),
            "wv": np.ascontiguousarray(Wv[:, hs:hs + DL]),
            "wo": np.ascontiguousarray(Wo[hs:hs + DL, :]),
            "bq": np.ascontiguousarray(bq[hs:hs + DL]),
            "bk": np.ascontiguousarray(bk[hs:hs + DL]),
            "bv": np.ascontiguousarray(bv[hs:hs + DL]),
            "masks": masks_np,
        })

    res = bass_utils.run_bass_kernel_spmd(nc, in_maps, core_ids=list(range(NCORES)))

    out = np.empty((B, S, D), np.float32)
    for b_ in range(B):
        acc = res.results[b_ * 4]["po"].astype(np.float32)
        for g in range(1, 4):
            acc = acc + res.results[b_ * 4 + g]["po"]
        out[b_] = acc.T + bo
    return out
